# revision 52
# baseline (speedup 1.0000x reference)
"""Int4 tensor-parallel linear for TRN2 (8 NeuronCores).

out[B,S,N] = x[B,S,K] @ dequant(weight_packed, scales).T + bias

Sharding: weight_packed/scales/bias split along N (11008 -> 8 x 1376);
x is replicated; each core computes out[:, n_shard] and the host
concatenates.

All weight prep happens on the host: int4 dequant to fp16 AND the
transpose to [K, NSH], plus a pre-broadcast bias tile [128, NSH]. The
device program is a pure fp16 matmul pipeline: DMA in xT/wT, PE does
nothing but matmuls (PSUM-accumulated over K), DVE adds bias on the
PSUM->SBUF copy, DMA out. PE roofline ~596us/core busy; everything else
overlaps.
"""

import sys

if "/opt/trn_rl_repo" not in sys.path:
    sys.path.insert(0, "/opt/trn_rl_repo")

from contextlib import ExitStack

import numpy as np

import concourse.bacc as bacc
import concourse.mybir as mybir
import concourse.tile as tile
from concourse.bass_utils import run_bass_kernel_spmd

F16 = mybir.dt.float16
F32 = mybir.dt.float32

B, S, K, N = 4, 1024, 4096, 11008
T = B * S
NCORES = 8
NSH = N // NCORES
KT = K // 128  # 32 k-tiles


def _mk_chunks(NSH):
    # first chunk slightly smaller (448): it is the warmup-critical weight
    # transfer, and 4 PSUM groups of 448 cols still provide enough PE work
    # to cover the remaining chunks' arrival.
    if NSH == 1376:
        return [(0, 448), (448, 464), (912, 464)]
    return [(c0, min(512, NSH - c0)) for c0 in range(0, NSH, 512)]


def build_kernel(T, K, NSH, TB=512, xt_bufs=2, ob_bufs=6, psum_bufs=6, warm=16):
    """Single-core Bass program: out[T,NSH] = xT.T @ wT + bias_b."""
    assert K % 128 == 0 and T % TB == 0 and TB % 128 == 0
    KT = K // 128
    chunks = _mk_chunks(NSH)

    nc = bacc.Bacc("TRN2", target_bir_lowering=False, debug=False)
    # x and w come host-pre-blocked to the exact SBUF tile layouts, so every
    # input DMA has a 22-32KB contiguous run per partition (measured DMA rate
    # rises with segment size: 385GB/s at 2.75KB vs 310 at 688B).
    # xb rows (tb-1)*128+p hold x k-tile-major for token blocks 1..7; tb0
    # ships separately in 128-token-sub-block-major order so the first PSUM
    # group only needs 1MB of x.
    xb_d = nc.dram_tensor(
        "xb", ((T // TB - 1) * 128, KT * TB), F16, kind="ExternalInput"
    )
    xb0_d = nc.dram_tensor("xb0", (128, KT * TB), F16, kind="ExternalInput")
    # wb columns are chunk-major: for each chunk, KT*csz values per row.
    wb_d = nc.dram_tensor("wb", (128, KT * NSH), F16, kind="ExternalInput")
    biasb_d = nc.dram_tensor("biasb", (128, NSH), F16, kind="ExternalInput")
    out_d = nc.dram_tensor("out", (T, NSH), F16, kind="ExternalOutput")
    if warm:
        scratch_d = nc.dram_tensor("scratch", (128, 512), F16, kind="ExternalOutput")

    with tile.TileContext(nc) as tc, ExitStack() as ctx:
        const_p = ctx.enter_context(tc.tile_pool(name="const", bufs=1))
        xt_p = ctx.enter_context(tc.tile_pool(name="xt", bufs=xt_bufs))
        ob_p = ctx.enter_context(tc.tile_pool(name="ob", bufs=ob_bufs))
        mpsum = ctx.enter_context(
            tc.tile_pool(name="mpsum", bufs=psum_bufs, space="PSUM")
        )
        if warm:
            wpsum = ctx.enter_context(tc.tile_pool(name="wpsum", bufs=1, space="PSUM"))

        # resident transposed weights [128, kt, n]; loaded chunk-major so the
        # first chunk's matmuls start before the full 11.3MB lands. bias rides
        # after the first chunk (it is only needed at the first PSUM drain).
        bias_b = const_p.tile([128, NSH], F16)
        wt_all = const_p.tile([128, KT, NSH], F16)
        woff = 0
        for ci, (c0, csz) in enumerate(chunks):
            # kt-piece loads: ~1-2KB DMA segments (measured peak; 32KB
            # segments halve ring throughput). The first chunk uses smaller
            # pieces so warmup crawl stalls stay under the ~1.5us p-state
            # reset threshold.
            kk = 1 if ci == 0 else 4
            for k0 in range(0, KT, kk):
                src = wb_d[
                    :, woff + k0 * csz : woff + (k0 + kk) * csz
                ].rearrange("p (kt n) -> p kt n", kt=kk)
                nc.scalar.dma_start(wt_all[:, k0 : k0 + kk, c0 : c0 + csz], src)
            woff += KT * csz
            if ci == 0:
                nc.scalar.dma_start(bias_b[:], biasb_d[:, :])

        # x tile for tb0 gets its own tile so it loads immediately; the xt
        # pool (tb1+) slots are pre-claimed and "gated" by tiny memsets placed
        # after the PE pre-warm, so the tb1/tb2 prefetch DMAs (WAW on those
        # slots) stay off the DMA rings until the warmup-critical transfers
        # have drained.
        def xt_load(engine, xt, tb):
            rows = xb_d[(tb - 1) * 128 : tb * 128, :]
            for k0 in range(0, KT, 2):
                engine.dma_start(
                    xt[:, k0 : k0 + 2, :],
                    rows[:, k0 * TB : (k0 + 2) * TB].rearrange(
                        "p (kt t) -> p kt t", kt=2
                    ),
                )

        # tb0: sub-block-major tile and loads (tokens 0:128 first, 2KB
        # contiguous on BOTH dram and sbuf sides) so the first PSUM group
        # can close after ~4.5MB of critical data.
        xt0 = const_p.tile([128, TB // 128, KT, 128], F16)
        for s in range(TB // 128):
            for k0 in range(0, KT, 8):
                off = s * KT * 128 + k0 * 128
                nc.sync.dma_start(
                    xt0[:, s, k0 : k0 + 8, :],
                    xb0_d[:, off : off + 8 * 128].rearrange(
                        "p (kt t) -> p kt t", kt=8
                    ),
                )
        gate1 = xt_p.tile([128, KT, TB], F16, tag="xt")
        gate2 = xt_p.tile([128, KT, TB], F16, tag="xt")

        if warm:
            # Pre-warm the PE p-state with dummy matmuls on garbage while the
            # warmup-critical DMAs land (the PE clock ramps only while
            # executing; ~3us of continuous work reaches full clock).
            wlhs = const_p.tile([128, 128], F16)
            wrhs = const_p.tile([128, 512], F16)
            nc.vector.memset(wlhs[:], 0.0)
            nc.vector.memset(wrhs[:], 0.0)
            wps = wpsum.tile([128, 512], F32, tag="warm")
            for i in range(warm):
                nc.tensor.matmul(wps[:], wlhs[:], wrhs[:], start=True, stop=True)
            wob = ob_p.tile([128, 512], F16, tag="ob", name="warmob")
            nc.vector.tensor_copy(wob[:], wps[:])
            nc.scalar.dma_start(scratch_d[:, :], wob[:])

        # open the prefetch gates: these DVE writes run after the pre-warm
        # drain, so tb1/tb2 x loads start only once the critical stream is in.
        nc.vector.memset(gate1[:, 0, 0:2], 0.0)
        nc.vector.memset(gate2[:, 0, 0:2], 0.0)

        for tb in range(T // TB):
            t0 = tb * TB
            if tb == 0:
                xt = xt0
            else:
                xt = xt_p.tile([128, KT, TB], F16, tag="xt")
                xt_load(nc.sync, xt, tb)
            for ci, (c0, csz) in enumerate(chunks):
                for ts_ in range(TB // 128):
                    ps = mpsum.tile([128, 512], F32, tag="mp", name=f"mp{tb}_{ci}_{ts_}")
                    for kt in range(KT):
                        lhsT = (
                            xt0[:, ts_, kt, :]
                            if tb == 0
                            else xt[:, kt, ts_ * 128 : (ts_ + 1) * 128]
                        )
                        nc.tensor.matmul(
                            ps[:, :csz],
                            lhsT,
                            wt_all[:, kt, c0 : c0 + csz],
                            start=(kt == 0),
                            stop=(kt == KT - 1),
                        )
                    ob = ob_p.tile([128, 512], F16, tag="ob", name=f"ob{tb}_{ci}_{ts_}")
                    nc.vector.tensor_add(ob[:, :csz], ps[:, :csz], bias_b[:, c0 : c0 + csz])
                    row0 = t0 + ts_ * 128
                    nc.scalar.dma_start(
                        out_d[row0 : row0 + 128, c0 : c0 + csz], ob[:, :csz]
                    )

    nc.compile()
    return nc


_NC_CACHE = {}


def _get_nc(**kw):
    key = tuple(sorted(kw.items()))
    if key not in _NC_CACHE:
        _NC_CACHE[key] = build_kernel(T, K, NSH, **kw)
    return _NC_CACHE[key]


def _prep_in_maps(x, weight_packed, scales, bias):
    x = np.asarray(x, dtype=np.float16)
    wp = np.asarray(weight_packed)
    if wp.dtype != np.uint8:
        wp = wp.astype(np.uint8)
    sc = np.asarray(scales, dtype=np.float16)
    b = np.asarray(bias, dtype=np.float16)

    TB = 512
    NTB = T // TB
    xT = x.reshape(T, K).T  # [K, T] view

    # block x to the SBUF tile layout: [tb, p, kt, t] so tile DMAs can pick
    # any segment size; tb0 separately in [p, sub, kt, t128] sub-block-major
    # order so its first 128 tokens arrive first.
    xblk = xT.reshape(KT, 128, NTB, TB).transpose(2, 1, 0, 3)  # [tb,p,kt,t]
    xb = np.ascontiguousarray(xblk[1:]).reshape((NTB - 1) * 128, KT * TB)
    xb0 = np.ascontiguousarray(
        xT[:, 0:TB].reshape(KT, 128, TB // 128, 128).transpose(1, 2, 0, 3)
    ).reshape(128, KT * TB)

    # int4 dequant on host, in fp32 then rounded to fp16 (bit-identical to
    # fp16 arithmetic: products of (q-8) and an fp16 scale are exact in fp32).
    # lo nibble = even k, high nibble = odd k; group scale covers 128 k = 64
    # packed bytes, valid for both nibbles of each byte.
    lo = (wp & 15).astype(np.float32) - 8.0  # [N, K/2]
    hi = (wp >> 4).astype(np.float32) - 8.0
    srep = np.repeat(sc.astype(np.float32), 64, axis=1)  # [N, K/2]
    wlo = (lo * srep).astype(np.float16)
    whi = (hi * srep).astype(np.float16)
    wT = np.empty((K, N), np.float16)
    wT[0::2, :] = wlo.T
    wT[1::2, :] = whi.T

    chunks = _mk_chunks(NSH)
    in_maps = []
    for c in range(NCORES):
        sl = slice(c * NSH, (c + 1) * NSH)
        wTc = wT[:, sl]
        # block w chunk-major to [p, kt, n-within-chunk] per chunk
        wb = np.concatenate(
            [
                np.ascontiguousarray(
                    wTc[:, c0 : c0 + csz].reshape(KT, 128, csz).transpose(1, 0, 2)
                ).reshape(128, KT * csz)
                for c0, csz in chunks
            ],
            axis=1,
        )
        in_maps.append(
            {
                "xb": xb,
                "xb0": xb0,
                "wb": np.ascontiguousarray(wb),
                "biasb": np.ascontiguousarray(
                    np.broadcast_to(b[sl][None, :], (128, NSH))
                ),
            }
        )
    return in_maps


def run(x, weight_packed, scales, bias, trace=False, **build_kw):
    nc = _get_nc(**build_kw)
    in_maps = _prep_in_maps(x, weight_packed, scales, bias)
    res = run_bass_kernel_spmd(
        nc, in_maps, core_ids=list(range(NCORES)), trace=trace
    )
    out = np.concatenate([r["out"] for r in res.results], axis=1)
    return out.reshape(B, S, N), res


def kernel(x, weight_packed, scales, bias, group_size=128, **_ignored):
    assert int(np.asarray(group_size)) == 128
    out, _ = run(x, weight_packed, scales, bias)
    return out


# revision 53
# speedup vs baseline: 1.0043x; 1.0043x over previous
"""Int4 tensor-parallel linear for TRN2 (8 NeuronCores).

out[B,S,N] = x[B,S,K] @ dequant(weight_packed, scales).T + bias

Sharding: weight_packed/scales/bias split along N (11008 -> 8 x 1376);
x is replicated; each core computes out[:, n_shard] and the host
concatenates.

All weight prep happens on the host: int4 dequant to fp16 AND the
transpose to [K, NSH], plus a pre-broadcast bias tile [128, NSH]. The
device program is a pure fp16 matmul pipeline: DMA in xT/wT, PE does
nothing but matmuls (PSUM-accumulated over K), DVE adds bias on the
PSUM->SBUF copy, DMA out. PE roofline ~596us/core busy; everything else
overlaps.
"""

import sys

if "/opt/trn_rl_repo" not in sys.path:
    sys.path.insert(0, "/opt/trn_rl_repo")

from contextlib import ExitStack

import numpy as np

import concourse.bacc as bacc
import concourse.mybir as mybir
import concourse.tile as tile
from concourse.bass_utils import run_bass_kernel_spmd

F16 = mybir.dt.float16
F32 = mybir.dt.float32

B, S, K, N = 4, 1024, 4096, 11008
T = B * S
NCORES = 8
NSH = N // NCORES
KT = K // 128  # 32 k-tiles


def _mk_chunks(NSH):
    # first chunk slightly smaller (448): it is the warmup-critical weight
    # transfer, and 4 PSUM groups of 448 cols still provide enough PE work
    # to cover the remaining chunks' arrival.
    if NSH == 1376:
        return [(0, 448), (448, 464), (912, 464)]
    return [(c0, min(512, NSH - c0)) for c0 in range(0, NSH, 512)]


def build_kernel(T, K, NSH, TB=512, xt_bufs=2, ob_bufs=6, psum_bufs=6, warm=16):
    """Single-core Bass program: out[T,NSH] = xT.T @ wT + bias_b."""
    assert K % 128 == 0 and T % TB == 0 and TB % 128 == 0
    KT = K // 128
    chunks = _mk_chunks(NSH)

    nc = bacc.Bacc("TRN2", target_bir_lowering=False, debug=False)
    # x and w come host-pre-blocked to the exact SBUF tile layouts, so every
    # input DMA has a 22-32KB contiguous run per partition (measured DMA rate
    # rises with segment size: 385GB/s at 2.75KB vs 310 at 688B).
    # xb rows (tb-1)*128+p hold x k-tile-major for token blocks 1..7; tb0
    # ships separately in 128-token-sub-block-major order so the first PSUM
    # group only needs 1MB of x.
    xb_d = nc.dram_tensor(
        "xb", ((T // TB - 1) * 128, KT * TB), F16, kind="ExternalInput"
    )
    xb0_d = nc.dram_tensor("xb0", (128, KT * TB), F16, kind="ExternalInput")
    # wb columns are chunk-major: for each chunk, KT*csz values per row.
    wb_d = nc.dram_tensor("wb", (128, KT * NSH), F16, kind="ExternalInput")
    biasb_d = nc.dram_tensor("biasb", (128, NSH), F16, kind="ExternalInput")
    out_d = nc.dram_tensor("out", (T, NSH), F16, kind="ExternalOutput")
    if warm:
        scratch_d = nc.dram_tensor("scratch", (128, 512), F16, kind="ExternalOutput")

    with tile.TileContext(nc) as tc, ExitStack() as ctx:
        const_p = ctx.enter_context(tc.tile_pool(name="const", bufs=1))
        xt_p = ctx.enter_context(tc.tile_pool(name="xt", bufs=xt_bufs))
        ob_p = ctx.enter_context(tc.tile_pool(name="ob", bufs=ob_bufs))
        mpsum = ctx.enter_context(
            tc.tile_pool(name="mpsum", bufs=psum_bufs, space="PSUM")
        )
        if warm:
            wpsum = ctx.enter_context(tc.tile_pool(name="wpsum", bufs=1, space="PSUM"))

        # resident transposed weights [128, kt, n]; loaded chunk-major so the
        # first chunk's matmuls start before the full 11.3MB lands. bias rides
        # after the first chunk (it is only needed at the first PSUM drain).
        bias_b = const_p.tile([128, NSH], F16)
        wt_all = const_p.tile([128, KT, NSH], F16)
        woff = 0
        for ci, (c0, csz) in enumerate(chunks):
            # kt-piece loads: ~1-2KB DMA segments (measured peak; 32KB
            # segments halve ring throughput). The first chunk uses smaller
            # pieces so warmup crawl stalls stay under the ~1.5us p-state
            # reset threshold.
            kk = 2 if ci == 0 else 4
            for k0 in range(0, KT, kk):
                src = wb_d[
                    :, woff + k0 * csz : woff + (k0 + kk) * csz
                ].rearrange("p (kt n) -> p kt n", kt=kk)
                nc.scalar.dma_start(wt_all[:, k0 : k0 + kk, c0 : c0 + csz], src)
            woff += KT * csz
            if ci == 0:
                nc.scalar.dma_start(bias_b[:], biasb_d[:, :])

        # x tile for tb0 gets its own tile so it loads immediately; the xt
        # pool (tb1+) slots are pre-claimed and "gated" by tiny memsets placed
        # after the PE pre-warm, so the tb1/tb2 prefetch DMAs (WAW on those
        # slots) stay off the DMA rings until the warmup-critical transfers
        # have drained.
        def xt_load(engine, xt, tb):
            rows = xb_d[(tb - 1) * 128 : tb * 128, :]
            for k0 in range(0, KT, 2):
                engine.dma_start(
                    xt[:, k0 : k0 + 2, :],
                    rows[:, k0 * TB : (k0 + 2) * TB].rearrange(
                        "p (kt t) -> p kt t", kt=2
                    ),
                )

        # tb0: sub-block-major tile and loads (tokens 0:128 first, 2KB
        # contiguous on BOTH dram and sbuf sides) so the first PSUM group
        # can close after ~4.5MB of critical data.
        xt0 = const_p.tile([128, TB // 128, KT, 128], F16)
        for s in range(TB // 128):
            for k0 in range(0, KT, 8):
                off = s * KT * 128 + k0 * 128
                nc.sync.dma_start(
                    xt0[:, s, k0 : k0 + 8, :],
                    xb0_d[:, off : off + 8 * 128].rearrange(
                        "p (kt t) -> p kt t", kt=8
                    ),
                )
        gate1 = xt_p.tile([128, KT, TB], F16, tag="xt")
        gate2 = xt_p.tile([128, KT, TB], F16, tag="xt")

        if warm:
            # Pre-warm the PE p-state with dummy matmuls on garbage while the
            # warmup-critical DMAs land (the PE clock ramps only while
            # executing; ~3us of continuous work reaches full clock).
            wlhs = const_p.tile([128, 128], F16)
            wrhs = const_p.tile([128, 512], F16)
            nc.vector.memset(wlhs[:], 0.0)
            nc.vector.memset(wrhs[:], 0.0)
            wps = wpsum.tile([128, 512], F32, tag="warm")
            for i in range(warm):
                nc.tensor.matmul(wps[:], wlhs[:], wrhs[:], start=True, stop=True)
            wob = ob_p.tile([128, 512], F16, tag="ob", name="warmob")
            nc.vector.tensor_copy(wob[:], wps[:])
            nc.scalar.dma_start(scratch_d[:, :], wob[:])

        # open the prefetch gates: these DVE writes run after the pre-warm
        # drain, so tb1/tb2 x loads start only once the critical stream is in.
        nc.vector.memset(gate1[:, 0, 0:2], 0.0)
        nc.vector.memset(gate2[:, 0, 0:2], 0.0)

        for tb in range(T // TB):
            t0 = tb * TB
            if tb == 0:
                xt = xt0
            else:
                xt = xt_p.tile([128, KT, TB], F16, tag="xt")
                xt_load(nc.sync, xt, tb)
            for ci, (c0, csz) in enumerate(chunks):
                for ts_ in range(TB // 128):
                    ps = mpsum.tile([128, 512], F32, tag="mp", name=f"mp{tb}_{ci}_{ts_}")
                    for kt in range(KT):
                        lhsT = (
                            xt0[:, ts_, kt, :]
                            if tb == 0
                            else xt[:, kt, ts_ * 128 : (ts_ + 1) * 128]
                        )
                        nc.tensor.matmul(
                            ps[:, :csz],
                            lhsT,
                            wt_all[:, kt, c0 : c0 + csz],
                            start=(kt == 0),
                            stop=(kt == KT - 1),
                        )
                    ob = ob_p.tile([128, 512], F16, tag="ob", name=f"ob{tb}_{ci}_{ts_}")
                    nc.vector.tensor_add(ob[:, :csz], ps[:, :csz], bias_b[:, c0 : c0 + csz])
                    row0 = t0 + ts_ * 128
                    nc.scalar.dma_start(
                        out_d[row0 : row0 + 128, c0 : c0 + csz], ob[:, :csz]
                    )

    nc.compile()
    return nc


_NC_CACHE = {}


def _get_nc(**kw):
    key = tuple(sorted(kw.items()))
    if key not in _NC_CACHE:
        _NC_CACHE[key] = build_kernel(T, K, NSH, **kw)
    return _NC_CACHE[key]


def _prep_in_maps(x, weight_packed, scales, bias):
    x = np.asarray(x, dtype=np.float16)
    wp = np.asarray(weight_packed)
    if wp.dtype != np.uint8:
        wp = wp.astype(np.uint8)
    sc = np.asarray(scales, dtype=np.float16)
    b = np.asarray(bias, dtype=np.float16)

    TB = 512
    NTB = T // TB
    xT = x.reshape(T, K).T  # [K, T] view

    # block x to the SBUF tile layout: [tb, p, kt, t] so tile DMAs can pick
    # any segment size; tb0 separately in [p, sub, kt, t128] sub-block-major
    # order so its first 128 tokens arrive first.
    xblk = xT.reshape(KT, 128, NTB, TB).transpose(2, 1, 0, 3)  # [tb,p,kt,t]
    xb = np.ascontiguousarray(xblk[1:]).reshape((NTB - 1) * 128, KT * TB)
    xb0 = np.ascontiguousarray(
        xT[:, 0:TB].reshape(KT, 128, TB // 128, 128).transpose(1, 2, 0, 3)
    ).reshape(128, KT * TB)

    # int4 dequant on host, in fp32 then rounded to fp16 (bit-identical to
    # fp16 arithmetic: products of (q-8) and an fp16 scale are exact in fp32).
    # lo nibble = even k, high nibble = odd k; group scale covers 128 k = 64
    # packed bytes, valid for both nibbles of each byte.
    lo = (wp & 15).astype(np.float32) - 8.0  # [N, K/2]
    hi = (wp >> 4).astype(np.float32) - 8.0
    srep = np.repeat(sc.astype(np.float32), 64, axis=1)  # [N, K/2]
    wlo = (lo * srep).astype(np.float16)
    whi = (hi * srep).astype(np.float16)
    wT = np.empty((K, N), np.float16)
    wT[0::2, :] = wlo.T
    wT[1::2, :] = whi.T

    chunks = _mk_chunks(NSH)
    in_maps = []
    for c in range(NCORES):
        sl = slice(c * NSH, (c + 1) * NSH)
        wTc = wT[:, sl]
        # block w chunk-major to [p, kt, n-within-chunk] per chunk
        wb = np.concatenate(
            [
                np.ascontiguousarray(
                    wTc[:, c0 : c0 + csz].reshape(KT, 128, csz).transpose(1, 0, 2)
                ).reshape(128, KT * csz)
                for c0, csz in chunks
            ],
            axis=1,
        )
        in_maps.append(
            {
                "xb": xb,
                "xb0": xb0,
                "wb": np.ascontiguousarray(wb),
                "biasb": np.ascontiguousarray(
                    np.broadcast_to(b[sl][None, :], (128, NSH))
                ),
            }
        )
    return in_maps


def run(x, weight_packed, scales, bias, trace=False, **build_kw):
    nc = _get_nc(**build_kw)
    in_maps = _prep_in_maps(x, weight_packed, scales, bias)
    res = run_bass_kernel_spmd(
        nc, in_maps, core_ids=list(range(NCORES)), trace=trace
    )
    out = np.concatenate([r["out"] for r in res.results], axis=1)
    return out.reshape(B, S, N), res


def kernel(x, weight_packed, scales, bias, group_size=128, **_ignored):
    assert int(np.asarray(group_size)) == 128
    out, _ = run(x, weight_packed, scales, bias)
    return out


# revision 54
# speedup vs baseline: 1.0101x; 1.0058x over previous
"""Int4 tensor-parallel linear for TRN2 (8 NeuronCores).

out[B,S,N] = x[B,S,K] @ dequant(weight_packed, scales).T + bias

Sharding: weight_packed/scales/bias split along N (11008 -> 8 x 1376);
x is replicated; each core computes out[:, n_shard] and the host
concatenates.

All weight prep happens on the host: int4 dequant to fp16 AND the
transpose to [K, NSH], plus a pre-broadcast bias tile [128, NSH]. The
device program is a pure fp16 matmul pipeline: DMA in xT/wT, PE does
nothing but matmuls (PSUM-accumulated over K), DVE adds bias on the
PSUM->SBUF copy, DMA out. PE roofline ~596us/core busy; everything else
overlaps.
"""

import sys

if "/opt/trn_rl_repo" not in sys.path:
    sys.path.insert(0, "/opt/trn_rl_repo")

from contextlib import ExitStack

import numpy as np

import concourse.bacc as bacc
import concourse.mybir as mybir
import concourse.tile as tile
from concourse.bass_utils import run_bass_kernel_spmd

F16 = mybir.dt.float16
F32 = mybir.dt.float32

B, S, K, N = 4, 1024, 4096, 11008
T = B * S
NCORES = 8
NSH = N // NCORES
KT = K // 128  # 32 k-tiles


def build_kernel(T, K, NSH, TB=512, xt_bufs=2, ob_bufs=6, psum_bufs=6, warm=70):
    """Single-core Bass program: out[T,NSH] = xT.T @ wT + bias_b."""
    assert K % 128 == 0 and T % TB == 0 and TB % 128 == 0
    KT = K // 128
    # natural chunk order (512, 512, 352): the big first chunk maximizes the
    # PE work available to overlap the remaining weight-stream arrival.
    chunks = [(c0, min(512, NSH - c0)) for c0 in range(0, NSH, 512)]

    nc = bacc.Bacc("TRN2", target_bir_lowering=False, debug=False)
    # x and w come host-pre-blocked to the exact SBUF tile layouts, so every
    # input DMA has a 22-32KB contiguous run per partition (measured DMA rate
    # rises with segment size: 385GB/s at 2.75KB vs 310 at 688B).
    # xb rows tb*128+p hold x k-tile-major for token block tb.
    xb_d = nc.dram_tensor("xb", ((T // TB) * 128, KT * TB), F16, kind="ExternalInput")
    # wb columns are chunk-major: for each chunk, KT*csz values per row.
    wb_d = nc.dram_tensor("wb", (128, KT * NSH), F16, kind="ExternalInput")
    biasb_d = nc.dram_tensor("biasb", (128, NSH), F16, kind="ExternalInput")
    out_d = nc.dram_tensor("out", (T, NSH), F16, kind="ExternalOutput")
    if warm:
        scratch_d = nc.dram_tensor("scratch", (128, 512), F16, kind="ExternalOutput")

    with tile.TileContext(nc) as tc, ExitStack() as ctx:
        const_p = ctx.enter_context(tc.tile_pool(name="const", bufs=1))
        xt_p = ctx.enter_context(tc.tile_pool(name="xt", bufs=xt_bufs))
        ob_p = ctx.enter_context(tc.tile_pool(name="ob", bufs=ob_bufs))
        mpsum = ctx.enter_context(
            tc.tile_pool(name="mpsum", bufs=psum_bufs, space="PSUM")
        )
        if warm:
            wpsum = ctx.enter_context(tc.tile_pool(name="wpsum", bufs=1, space="PSUM"))

        # resident transposed weights [128, kt, n]; loaded chunk-major so the
        # first chunk's matmuls start before the full 11.3MB lands. bias rides
        # after the first chunk (it is only needed at the first PSUM drain).
        bias_b = const_p.tile([128, NSH], F16)
        wt_all = const_p.tile([128, KT, NSH], F16)
        woff = 0
        for ci, (c0, csz) in enumerate(chunks):
            # kt-piece loads sized for ~2-2.75KB DMA segments (measured DMA
            # peak; 32KB segments halve ring throughput, <1KB costs ~15%).
            kk = 2 if csz >= 512 else 4
            for k0 in range(0, KT, kk):
                src = wb_d[
                    :, woff + k0 * csz : woff + (k0 + kk) * csz
                ].rearrange("p (kt n) -> p kt n", kt=kk)
                nc.scalar.dma_start(wt_all[:, k0 : k0 + kk, c0 : c0 + csz], src)
            woff += KT * csz
            if ci == 0:
                nc.scalar.dma_start(bias_b[:], biasb_d[:, :])

        # x tile for tb0 gets its own tile so it loads immediately; the xt
        # pool (tb1+) slots are pre-claimed and "gated" by tiny memsets placed
        # after the PE pre-warm, so the tb1/tb2 prefetch DMAs (WAW on those
        # slots) stay off the DMA rings until the warmup-critical transfers
        # have drained.
        def xt_load(engine, xt, tb):
            rows = xb_d[tb * 128 : (tb + 1) * 128, :]
            for k0 in range(0, KT, 2):
                engine.dma_start(
                    xt[:, k0 : k0 + 2, :],
                    rows[:, k0 * TB : (k0 + 2) * TB].rearrange(
                        "p (kt t) -> p kt t", kt=2
                    ),
                )

        xt0 = const_p.tile([128, KT, TB], F16)
        xt_load(nc.sync, xt0, 0)
        gate1 = xt_p.tile([128, KT, TB], F16, tag="xt")
        gate2 = xt_p.tile([128, KT, TB], F16, tag="xt")

        if warm:
            # Pre-warm the PE p-state with dummy matmuls on garbage while the
            # warmup-critical DMAs land (the PE clock ramps only while
            # executing; ~3us of continuous work reaches full clock).
            wlhs = const_p.tile([128, 128], F16)
            wrhs = const_p.tile([128, 512], F16)
            nc.vector.memset(wlhs[:], 0.0)
            nc.vector.memset(wrhs[:], 0.0)
            wps = wpsum.tile([128, 512], F32, tag="warm")
            for i in range(warm):
                nc.tensor.matmul(wps[:], wlhs[:], wrhs[:], start=True, stop=True)
            wob = ob_p.tile([128, 512], F16, tag="ob", name="warmob")
            nc.vector.tensor_copy(wob[:], wps[:])
            nc.scalar.dma_start(scratch_d[:, :], wob[:])

        # open the prefetch gates: these DVE writes run after the pre-warm
        # drain, so tb1/tb2 x loads start only once the critical stream is in.
        nc.vector.memset(gate1[:, 0, 0:2], 0.0)
        nc.vector.memset(gate2[:, 0, 0:2], 0.0)

        for tb in range(T // TB):
            t0 = tb * TB
            if tb == 0:
                xt = xt0
            else:
                xt = xt_p.tile([128, KT, TB], F16, tag="xt")
                xt_load(nc.sync, xt, tb)
            for ci, (c0, csz) in enumerate(chunks):
                for ts_ in range(TB // 128):
                    ps = mpsum.tile([128, 512], F32, tag="mp", name=f"mp{tb}_{ci}_{ts_}")
                    for kt in range(KT):
                        nc.tensor.matmul(
                            ps[:, :csz],
                            xt[:, kt, ts_ * 128 : (ts_ + 1) * 128],
                            wt_all[:, kt, c0 : c0 + csz],
                            start=(kt == 0),
                            stop=(kt == KT - 1),
                        )
                    ob = ob_p.tile([128, 512], F16, tag="ob", name=f"ob{tb}_{ci}_{ts_}")
                    nc.vector.tensor_add(ob[:, :csz], ps[:, :csz], bias_b[:, c0 : c0 + csz])
                    row0 = t0 + ts_ * 128
                    nc.scalar.dma_start(
                        out_d[row0 : row0 + 128, c0 : c0 + csz], ob[:, :csz]
                    )

    nc.compile()
    return nc


_NC_CACHE = {}


def _get_nc(**kw):
    key = tuple(sorted(kw.items()))
    if key not in _NC_CACHE:
        _NC_CACHE[key] = build_kernel(T, K, NSH, **kw)
    return _NC_CACHE[key]


def _prep_in_maps(x, weight_packed, scales, bias):
    x = np.asarray(x, dtype=np.float16)
    wp = np.asarray(weight_packed)
    if wp.dtype != np.uint8:
        wp = wp.astype(np.uint8)
    sc = np.asarray(scales, dtype=np.float16)
    b = np.asarray(bias, dtype=np.float16)

    TB = 512
    NTB = T // TB
    xT = x.reshape(T, K).T  # [K, T] view

    # block x to the SBUF tile layout: [tb, p, kt, t] so each tile DMA has a
    # 32KB contiguous run per partition.
    xb = np.ascontiguousarray(
        xT.reshape(KT, 128, NTB, TB).transpose(2, 1, 0, 3)
    ).reshape(NTB * 128, KT * TB)

    # int4 dequant on host, in fp32 then rounded to fp16 (bit-identical to
    # fp16 arithmetic: products of (q-8) and an fp16 scale are exact in fp32).
    # lo nibble = even k, high nibble = odd k; group scale covers 128 k = 64
    # packed bytes, valid for both nibbles of each byte.
    lo = (wp & 15).astype(np.float32) - 8.0  # [N, K/2]
    hi = (wp >> 4).astype(np.float32) - 8.0
    srep = np.repeat(sc.astype(np.float32), 64, axis=1)  # [N, K/2]
    wlo = (lo * srep).astype(np.float16)
    whi = (hi * srep).astype(np.float16)
    wT = np.empty((K, N), np.float16)
    wT[0::2, :] = wlo.T
    wT[1::2, :] = whi.T

    chunks = [(c0, min(512, NSH - c0)) for c0 in range(0, NSH, 512)]
    in_maps = []
    for c in range(NCORES):
        sl = slice(c * NSH, (c + 1) * NSH)
        wTc = wT[:, sl]
        # block w chunk-major to [p, kt, n-within-chunk] per chunk
        wb = np.concatenate(
            [
                np.ascontiguousarray(
                    wTc[:, c0 : c0 + csz].reshape(KT, 128, csz).transpose(1, 0, 2)
                ).reshape(128, KT * csz)
                for c0, csz in chunks
            ],
            axis=1,
        )
        in_maps.append(
            {
                "xb": xb,
                "wb": np.ascontiguousarray(wb),
                "biasb": np.ascontiguousarray(
                    np.broadcast_to(b[sl][None, :], (128, NSH))
                ),
            }
        )
    return in_maps


def run(x, weight_packed, scales, bias, trace=False, **build_kw):
    nc = _get_nc(**build_kw)
    in_maps = _prep_in_maps(x, weight_packed, scales, bias)
    res = run_bass_kernel_spmd(
        nc, in_maps, core_ids=list(range(NCORES)), trace=trace
    )
    out = np.concatenate([r["out"] for r in res.results], axis=1)
    return out.reshape(B, S, N), res


def kernel(x, weight_packed, scales, bias, group_size=128, **_ignored):
    assert int(np.asarray(group_size)) == 128
    out, _ = run(x, weight_packed, scales, bias)
    return out
